# revision 1
# baseline (speedup 1.0000x reference)
"""Bass/Tile kernel for a 4-layer dense transformer (prefill) on 8 TRN2 cores.

Parallelization: 2-way data parallel (batch) x 4-way tensor parallel.
Groups: cores [0,1,2,3] handle batch 0, [4,5,6,7] batch 1.
Within a group (rank r = core % 4):
  - attention: heads r*4..r*4+3  (feature cols r*256..(r+1)*256)
  - MLP: hidden cols r*1024..(r+1)*1024
  - vocab: cols r*8000..(r+1)*8000 of head_w
Activations are kept TRANSPOSED on device: [feature(partition), token(free)].
Residual stream x is fp32; matmul inputs are bf16 (fp32 PSUM accumulation).
Per layer: AllGather(attn-out bf16), AllGather(attn-delta fp32),
AllGather(mlp-hidden bf16), AllGather(mlp-delta fp32).
Final logits are computed in natural [token, vocab] layout and written out
per-core as [1024, 8000]; the host concatenates.
"""

import sys
import types

import numpy as np


def _install_ntff_shim():
    """Register the NTFF profiling hook that trn_boot skipped (the image's
    antenv package lacks the axon_hooks submodule)."""
    if "antenv.axon_hooks" in sys.modules:
        return
    try:
        import trn_agent_boot.trn_boot as tb
        hook = tb._ntff_profile_via_ctypes("/opt/axon/libaxon_pjrt.so")
    except Exception:
        hook = None
    mod = types.ModuleType("antenv.axon_hooks")
    _h = [hook]
    mod.get_axon_ntff_profile_hook = lambda: _h[0]
    mod.set_axon_ntff_profile_hook = lambda h: _h.__setitem__(0, h)
    sys.modules["antenv.axon_hooks"] = mod
    try:
        import antenv
        antenv.axon_hooks = mod
    except Exception:
        pass


_install_ntff_shim()

import ml_dtypes
import concourse.bass as bass
import concourse.mybir as mybir
import concourse.tile as tile
from concourse import bacc
from concourse.bass_utils import run_bass_kernel_spmd

BF = mybir.dt.bfloat16
F32 = mybir.dt.float32
AL = mybir.AluOpType
AF = mybir.ActivationFunctionType

# Model sizes (full problem, hardcoded per contract).
CFG = dict(
    B=2, S=1024, V=32000, D=1024, H=16, L=4, EPS=1e-5,
    TP=4,            # tensor-parallel width (group size)
    gelu_sim=False,  # CoreSim lacks Gelu; use sigmoid-based stand-in
)

N_CORES = 8
GROUPS = [[0, 1, 2, 3], [4, 5, 6, 7]]


def build_program(cfg=None):
    """Build the SPMD Bass program (identical on all 8 cores)."""
    c = dict(CFG)
    if cfg:
        c.update(cfg)
    B, S, V, D, H, L = c["B"], c["S"], c["V"], c["D"], c["H"], c["L"]
    EPS, TP = c["EPS"], c["TP"]
    T = S                    # tokens per group (one batch element)
    DK = D // H              # head dim (64)
    HL = H // TP             # heads per core (4)
    DSH = D // TP            # attention/delta feature shard (256)
    DF = 4 * D
    DFS = DF // TP           # mlp hidden shard (1024)
    VSH = V // TP            # vocab shard (8000)
    KT = D // 128            # feature k-tiles (8)
    KTF = DF // 128          # mlp k-tiles (32)
    NCH = max(1, T // 512)   # token chunks of <=512
    TCH = min(512, T)        # token chunk size
    MSH = DSH // 128         # m-tiles of a DSH-wide output (2)
    TKT = T // 128           # key-token tiles (8)
    VCH = 500                # vocab chunk
    NV = VSH // VCH          # vocab n-chunks (16)
    TT = T // 128            # token tiles (8)
    assert T % 128 == 0 and D % 128 == 0 and DSH % 128 == 0
    assert VSH % NV == 0 and VCH <= 512

    groups = [[g * TP + r for r in range(TP)] for g in range(N_CORES // TP)]

    nc = bacc.Bacc("TRN2", target_bir_lowering=False, debug=False,
                   num_devices=N_CORES)

    # ---- DRAM parameters (per-core shards fed via in_maps) ----
    xT0 = nc.dram_tensor("xT0", [D, T], F32, kind="ExternalInput")
    wq = nc.dram_tensor("wq", [L, D, DSH], BF, kind="ExternalInput")
    wk = nc.dram_tensor("wk", [L, D, DSH], BF, kind="ExternalInput")
    wv = nc.dram_tensor("wv", [L, D, DSH], BF, kind="ExternalInput")
    wo = nc.dram_tensor("wo", [L, D, DSH], BF, kind="ExternalInput")
    w1 = nc.dram_tensor("w1", [L, D, DFS], BF, kind="ExternalInput")
    w2 = nc.dram_tensor("w2", [L, DF, DSH], BF, kind="ExternalInput")
    b1 = nc.dram_tensor("b1", [L, DFS], F32, kind="ExternalInput")
    b2 = nc.dram_tensor("b2", [L, D], F32, kind="ExternalInput")
    g1 = nc.dram_tensor("g1", [L, D], F32, kind="ExternalInput")
    be1 = nc.dram_tensor("be1", [L, D], F32, kind="ExternalInput")
    g2 = nc.dram_tensor("g2", [L, D], F32, kind="ExternalInput")
    be2 = nc.dram_tensor("be2", [L, D], F32, kind="ExternalInput")
    gf = nc.dram_tensor("gf", [1, D], F32, kind="ExternalInput")
    bef = nc.dram_tensor("bef", [1, D], F32, kind="ExternalInput")
    hw = nc.dram_tensor("hw", [D, VSH], BF, kind="ExternalInput")
    logits = nc.dram_tensor("logits", [T, VSH], F32, kind="ExternalOutput")

    with tile.TileContext(nc) as tc:
        _build_tc(nc, tc, locals())
    nc.compile()
    return nc


def _build_tc(nc, tc, v):
    """Emit the tile program. `v` is the name->value dict from build_program."""
    (B, T, D, L, EPS, TP, DK, HL, DSH, DF, DFS, VSH, KT, KTF, NCH, TCH,
     MSH, TKT, NV, VCH, TT, groups) = (
        v["B"], v["T"], v["D"], v["L"], v["EPS"], v["TP"], v["DK"], v["HL"],
        v["DSH"], v["DF"], v["DFS"], v["VSH"], v["KT"], v["KTF"], v["NCH"],
        v["TCH"], v["MSH"], v["TKT"], v["NV"], v["VCH"], v["TT"], v["groups"])
    xT0, wq, wk, wv, wo, w1, w2 = (v["xT0"], v["wq"], v["wk"], v["wv"],
                                   v["wo"], v["w1"], v["w2"])
    b1d, b2d, g1d, be1d, g2d, be2d, gfd, befd = (
        v["b1"], v["b2"], v["g1"], v["be1"], v["g2"], v["be2"], v["gf"],
        v["bef"])
    hwd, logits = v["hw"], v["logits"]

    import contextlib
    ctx = contextlib.ExitStack()

    # ---------------- pools ----------------
    sing = ctx.enter_context(tc.tile_pool(name="sing", bufs=1))
    wts = ctx.enter_context(tc.tile_pool(name="wts", bufs=1))
    w1s = ctx.enter_context(tc.tile_pool(name="w1s", bufs=3))
    hwp = ctx.enter_context(tc.tile_pool(name="hwp", bufs=9))
    hp = ctx.enter_context(tc.tile_pool(name="hp", bufs=1))
    qkp = ctx.enter_context(tc.tile_pool(name="qkp", bufs=1))
    scr = ctx.enter_context(tc.tile_pool(name="scr", bufs=2))
    expp = ctx.enter_context(tc.tile_pool(name="expp", bufs=8))
    otp = ctx.enter_context(tc.tile_pool(name="otp", bufs=1))
    agb = ctx.enter_context(tc.tile_pool(name="agb", bufs=3))   # bf16 AG reads
    agf = ctx.enter_context(tc.tile_pool(name="agf", bufs=2))   # f32 AG reads
    dshp = ctx.enter_context(tc.tile_pool(name="dshp", bufs=1))
    up = ctx.enter_context(tc.tile_pool(name="up", bufs=3))
    lgp = ctx.enter_context(tc.tile_pool(name="lgp", bufs=2))
    tiny = ctx.enter_context(tc.tile_pool(name="tiny", bufs=2))
    rows1 = ctx.enter_context(tc.tile_pool(name="rows1", bufs=1))
    rows3 = ctx.enter_context(tc.tile_pool(name="rows3", bufs=3))
    rows2 = ctx.enter_context(tc.tile_pool(name="rows2", bufs=1))
    bcp = ctx.enter_context(tc.tile_pool(name="bcp", bufs=1))
    rbp = ctx.enter_context(tc.tile_pool(name="rbp", bufs=2))
    psmm = ctx.enter_context(tc.tile_pool(name="psmm", bufs=4, space="PSUM"))
    psaux = ctx.enter_context(tc.tile_pool(name="psaux", bufs=2, space="PSUM"))
    psst = ctx.enter_context(tc.tile_pool(name="psst", bufs=2, space="PSUM"))
    dram = ctx.enter_context(tc.tile_pool(name="dram", bufs=1, space="DRAM"))

    # ---------------- constants ----------------
    ones_col = sing.tile([128, 1], BF, name="ones_col")
    nc.vector.memset(ones_col, 1.0)
    ones_row = sing.tile([1, 128], BF, name="ones_row")
    nc.vector.memset(ones_row, 1.0)
    ones_row_f = sing.tile([1, 128], F32, name="ones_row_f")
    nc.vector.memset(ones_row_f, 1.0)
    eps_ap = sing.tile([1, 1], F32, name="eps_ap")
    nc.vector.memset(eps_ap, EPS)

    # ---------------- residual stream ----------------
    x = [sing.tile([128, T], F32, name=f"x{k}") for k in range(KT)]
    for k in range(KT):
        nc.sync.dma_start(out=x[k], in_=xT0[k * 128:(k + 1) * 128, :])

    # ---------------- layernorm ----------------
    def layernorm(x_tiles, grow_dram, brow_dram, name):
        """LN over the feature (partition) axis of transposed activations.
        Returns bf16 tiles h[kt] = LN(x)."""
        # per-partition gamma/beta columns: [128, KT]
        gcol = tiny.tile([128, KT], F32, name=f"g_{name}")
        bcol = tiny.tile([128, KT], F32, name=f"b_{name}")
        nc.sync.dma_start(out=gcol, in_=grow_dram.rearrange("(k p) -> p k", p=128))
        nc.sync.dma_start(out=bcol, in_=brow_dram.rearrange("(k p) -> p k", p=128))

        # stats: k-outer so xb/sq are transient. Per chunk one PSUM tile
        # holds sum at partition 0 and sumsq at partition 32.
        ps_st = [psst.tile([33, TCH], F32, name="ps_st", tag="ps_st")
                 for _ in range(NCH)]
        for k in range(KT):
            xbt = scr.tile([128, T], BF, name="xb", tag="xb")
            nc.vector.tensor_copy(xbt, x_tiles[k])
            sqt = scr.tile([128, T], BF, name="sq", tag="sq")
            nc.scalar.square(sqt, xbt)
            for ch in range(NCH):
                cs = slice(ch * TCH, (ch + 1) * TCH)
                nc.tensor.matmul(ps_st[ch][0:1, :], ones_col, xbt[:, cs],
                                 start=(k == 0), stop=(k == KT - 1))
                nc.tensor.matmul(ps_st[ch][32:33, :], ones_col, sqt[:, cs],
                                 start=(k == 0), stop=(k == KT - 1))
        # st_sb[0, 0:T]=sum, [0, T:2T]=sumsq
        st_sb = rows1.tile([1, 2 * T], F32, name=f"st_{name}", tag="st_sb")
        for ch in range(NCH):
            nc.vector.tensor_copy(st_sb[:, ch * TCH:(ch + 1) * TCH],
                                  ps_st[ch][0:1, :])
            nc.vector.tensor_copy(st_sb[:, T + ch * TCH:T + (ch + 1) * TCH],
                                  ps_st[ch][32:33, :])
        # moments
        mom = rows1.tile([1, 2 * T], F32, name=f"mom_{name}", tag="mom")
        nc.scalar.mul(mom, st_sb, 1.0 / D)      # [mean | E[x^2]]
        mean = mom[:, 0:T]
        msq = mom[:, T:2 * T]
        m2 = rows3.tile([1, T], F32, name=f"m2_{name}", tag="row1k")
        nc.vector.tensor_mul(m2, mean, mean)
        var = rows3.tile([1, T], F32, name=f"var_{name}", tag="row1k")
        nc.vector.tensor_tensor(out=var, in0=msq, in1=m2, op=AL.subtract)
        sd = rows3.tile([1, T], F32, name=f"sd_{name}", tag="row1k")
        nc.scalar.activation(sd, var, AF.Sqrt, bias=eps_ap)
        rstd = rows3.tile([1, T], F32, name=f"rstd_{name}", tag="row1k")
        nc.vector.reciprocal(rstd, sd)
        nmr = rows3.tile([1, T], F32, name=f"nmr_{name}", tag="row1k")
        nc.vector.tensor_mul(nmr, mean, rstd)
        nc.scalar.mul(nmr, nmr, -1.0)           # -mean*rstd
        # broadcast to [128, T] via K=1 outer-product matmuls (fp32)
        rstdB = bcp.tile([128, T], F32, name="rstdB", tag="rstdB")
        nmB = bcp.tile([128, T], F32, name="nmB", tag="nmB")
        for ch in range(NCH):
            cs = slice(ch * TCH, (ch + 1) * TCH)
            pb = psaux.tile([128, TCH], F32, name="pb", tag="aux")
            nc.tensor.matmul(pb, ones_row_f, rstd[:, cs], start=True,
                             stop=True)
            nc.scalar.copy(rstdB[:, cs], pb)
            pb2 = psaux.tile([128, TCH], F32, name="pb2", tag="aux")
            nc.tensor.matmul(pb2, ones_row_f, nmr[:, cs], start=True,
                             stop=True)
            nc.scalar.copy(nmB[:, cs], pb2)
        # apply: h = (x*rstdB + nmB)*g + b, output bf16
        h = []
        for k in range(KT):
            t1 = scr.tile([128, T], F32, name="lnt", tag="lnt")
            nc.vector.tensor_mul(t1, x_tiles[k], rstdB)
            t2 = scr.tile([128, T], BF, name="lnt2", tag="lnt2")
            nc.vector.tensor_tensor(out=t2, in0=t1, in1=nmB, op=AL.add)
            ht = hp.tile([128, T], BF, name=f"h{k}", tag=f"h{k}")
            nc.vector.tensor_scalar(
                out=ht, in0=t2, scalar1=gcol[:, k:k + 1],
                scalar2=bcol[:, k:k + 1], op0=AL.mult, op1=AL.add)
            h.append(ht)
        return h

    # ---------------- transformer layers ----------------
    for l in range(L):
        # -- weights for this layer --
        wqt = wts.tile([128, KT, DSH], BF, name="wqt", tag="wqt")
        wkt = wts.tile([128, KT, DSH], BF, name="wkt", tag="wkt")
        wvt = wts.tile([128, KT, DSH], BF, name="wvt", tag="wvt")
        wot = wts.tile([128, KT, DSH], BF, name="wot", tag="wot")
        for dst, src in ((wqt, wq), (wkt, wk), (wvt, wv), (wot, wo)):
            nc.sync.dma_start(
                out=dst, in_=src[l].rearrange("(k p) m -> p k m", p=128))
        b1col = tiny.tile([128, DFS // 128], F32, name="b1col", tag="b1col")
        nc.sync.dma_start(out=b1col, in_=b1d[l].rearrange("(k p) -> p k", p=128))
        b2col = tiny.tile([128, KT], F32, name="b2col", tag="b2col")
        nc.sync.dma_start(out=b2col, in_=b2d[l].rearrange("(k p) -> p k", p=128))

        # -- LN1 --
        h1 = layernorm(x, g1d[l], be1d[l], f"ln1_{l}")

        # -- QKV projections --
        # qT/kT: [DSH, T] transposed; v: natural [T, DSH] + ones column
        qT = [qkp.tile([128, T], BF, name=f"qT{m}", tag=f"qT{m}")
              for m in range(MSH)]
        kTt = [qkp.tile([128, T], BF, name=f"kT{m}", tag=f"kT{m}")
               for m in range(MSH)]
        for wt, dst in ((wqt, qT), (wkt, kTt)):
            pq = {}
            for m in range(MSH):
                for chn in range(NCH):
                    pq[(m, chn)] = psmm.tile([128, TCH], F32, name="ps",
                                             tag="mm")
            for k in range(KT):
                for m in range(MSH):
                    for chn in range(NCH):
                        cs = slice(chn * TCH, (chn + 1) * TCH)
                        nc.tensor.matmul(pq[(m, chn)],
                                         wt[:, k, m * 128:(m + 1) * 128],
                                         h1[k][:, cs],
                                         start=(k == 0), stop=(k == KT - 1))
            for m in range(MSH):
                for chn in range(NCH):
                    cs = slice(chn * TCH, (chn + 1) * TCH)
                    nc.vector.tensor_copy(dst[m][:, cs], pq[(m, chn)])
        vt = qkp.tile([128, TKT, HL, DK + 1], BF, name="vt", tag="vt")
        nc.vector.memset(vt[:, :, :, DK:DK + 1], 1.0)
        for t in range(TT):
            ps = psmm.tile([128, TCH], F32, name="psv", tag="mm")
            for k in range(KT):
                nc.tensor.matmul(ps[:, 0:DSH],
                                 h1[k][:, t * 128:(t + 1) * 128],
                                 wvt[:, k, :],
                                 start=(k == 0), stop=(k == KT - 1))
            nc.vector.tensor_copy(
                vt[:, t, :, 0:DK],
                ps[:, 0:DSH].rearrange("p (h d) -> p h d", h=HL))

        # -- attention per head --
        oT = [otp.tile([128, T], BF, name=f"oT{m}", tag=f"oT{m}")
              for m in range(MSH)]
        for hh in range(HL):
            mt = (hh * DK) // 128
            po = (hh * DK) % 128
            q_h = qT[mt][po:po + DK, :]
            k_h = kTt[mt][po:po + DK, :]
            for chn in range(NCH):
                cs = slice(chn * TCH, (chn + 1) * TCH)
                jmax = (chn + 1) * (TCH // 128)
                exps = []
                for j in range(jmax):
                    pss = psmm.tile([128, TCH], F32, name="pss", tag="mm")
                    nc.tensor.matmul(pss, k_h[:, j * 128:(j + 1) * 128],
                                     q_h[:, cs], start=True, stop=True)
                    et = expp.tile([128, TCH], BF, name="exp", tag="exp")
                    nc.scalar.activation(et, pss, AF.Exp, scale=0.125)
                    if j * 128 >= chn * TCH:
                        # diagonal block: zero where tk_global > tq_global
                        nc.gpsimd.affine_select(
                            out=et, in_=et, pattern=[[1, TCH]],
                            compare_op=AL.is_ge, fill=0.0,
                            base=chn * TCH - j * 128, channel_multiplier=-1)
                    exps.append(et)
                ps_o = psaux.tile([DK + 1, TCH], F32, name="ps_o", tag="aux")
                for j in range(jmax):
                    nc.tensor.matmul(ps_o, vt[:, j, hh, :], exps[j],
                                     start=(j == 0), stop=(j == jmax - 1))
                rec = rows2.tile([1, TCH], F32, name="rec", tag="rec")
                den = rows2.tile([1, TCH], F32, name="den", tag="den")
                nc.vector.tensor_copy(den, ps_o[DK:DK + 1, :])
                rsc = rows2.tile([1, TCH], F32, name="rsc", tag="rsc")
                nc.vector.reciprocal_approx_accurate(rec, den, rsc)
                recb = rows2.tile([1, TCH], BF, name="recb", tag="recb")
                nc.vector.tensor_copy(recb, rec)
                ps_r = psmm.tile([128, TCH], F32, name="ps_r", tag="mm")
                nc.tensor.matmul(ps_r[0:DK, :], ones_row[:, 0:DK], recb,
                                 start=True, stop=True)
                rb = rbp.tile([DK, TCH], BF, name="rb", tag="rb")
                nc.scalar.copy(rb, ps_r[0:DK, :])
                nc.vector.tensor_tensor(
                    out=oT[mt][po:po + DK, cs], in0=ps_o[0:DK, :], in1=rb,
                    op=AL.mult)

        # -- AllGather attention heads --
        o_in = dram.tile([DSH, T], BF, name=f"o_in{l}")
        o_out = dram.tile([TP * DSH, T], BF, name=f"o_out{l}")
        for m in range(MSH):
            nc.sync.dma_start(out=o_in[m * 128:(m + 1) * 128, :], in_=oT[m])
        nc.gpsimd.collective_compute(
            "AllGather", AL.bypass, replica_groups=groups,
            ins=[o_in.opt()], outs=[o_out.opt()])

        # -- attn out projection: d1 = o_full @ Wo[:, shard] --
        d1 = [dshp.tile([128, T], F32, name=f"d1_{m}", tag=f"dsh{m}")
              for m in range(MSH)]
        pso = {}
        for m in range(MSH):
            for chn in range(NCH):
                pso[(m, chn)] = psmm.tile([128, TCH], F32, name="psd1",
                                          tag="mm")
        for k in range(KT):
            of = agb.tile([128, T], BF, name="of", tag="agb")
            nc.sync.dma_start(out=of, in_=o_out[k * 128:(k + 1) * 128, :])
            for m in range(MSH):
                for chn in range(NCH):
                    cs = slice(chn * TCH, (chn + 1) * TCH)
                    nc.tensor.matmul(pso[(m, chn)],
                                     wot[:, k, m * 128:(m + 1) * 128],
                                     of[:, cs],
                                     start=(k == 0), stop=(k == KT - 1))
        for m in range(MSH):
            for chn in range(NCH):
                cs = slice(chn * TCH, (chn + 1) * TCH)
                nc.vector.tensor_copy(d1[m][:, cs], pso[(m, chn)])
        d1_in = dram.tile([DSH, T], F32, name=f"d1_in{l}")
        d1_out = dram.tile([TP * DSH, T], F32, name=f"d1_out{l}")
        for m in range(MSH):
            nc.sync.dma_start(out=d1_in[m * 128:(m + 1) * 128, :], in_=d1[m])
        nc.gpsimd.collective_compute(
            "AllGather", AL.bypass, replica_groups=groups,
            ins=[d1_in.opt()], outs=[d1_out.opt()])
        for k in range(KT):
            df = agf.tile([128, T], F32, name="df", tag="agf")
            nc.sync.dma_start(out=df, in_=d1_out[k * 128:(k + 1) * 128, :])
            nc.vector.tensor_tensor(out=x[k], in0=x[k], in1=df, op=AL.add)

        # -- LN2 + MLP --
        h2 = layernorm(x, g2d[l], be2d[l], f"ln2_{l}")
        u_in = dram.tile([DFS, T], BF, name=f"u_in{l}")
        u_out = dram.tile([TP * DFS, T], BF, name=f"u_out{l}")
        for m in range(DFS // 128):
            w1m = w1s.tile([128, KT, 128], BF, name="w1m", tag="w1m")
            nc.sync.dma_start(
                out=w1m,
                in_=w1[l][:, m * 128:(m + 1) * 128].rearrange(
                    "(k p) m -> p k m", p=128))
            ut = up.tile([128, T], BF, name="ut", tag="ut")
            pu = {}
            for chn in range(NCH):
                pu[chn] = psmm.tile([128, TCH], F32, name="psu", tag="mm")
            for k in range(KT):
                for chn in range(NCH):
                    cs = slice(chn * TCH, (chn + 1) * TCH)
                    nc.tensor.matmul(pu[chn], w1m[:, k, :], h2[k][:, cs],
                                     start=(k == 0), stop=(k == KT - 1))
            for chn in range(NCH):
                cs = slice(chn * TCH, (chn + 1) * TCH)
                ps = pu[chn]
                if v["c"].get("gelu_sim"):
                    # CoreSim stand-in: gelu(x) ~ x*sigmoid(1.702x)
                    u0 = scr.tile([128, TCH], F32, name="u0", tag="u0")
                    nc.vector.tensor_scalar_add(u0, ps, b1col[:, m:m + 1])
                    sg = scr.tile([128, TCH], F32, name="sg", tag="sg")
                    nc.scalar.activation(sg, u0, AF.Sigmoid, scale=1.702)
                    nc.vector.tensor_mul(ut[:, cs], u0, sg)
                else:
                    nc.scalar.activation(ut[:, cs], ps, AF.Gelu,
                                         bias=b1col[:, m:m + 1])
            nc.sync.dma_start(out=u_in[m * 128:(m + 1) * 128, :], in_=ut)
        nc.gpsimd.collective_compute(
            "AllGather", AL.bypass, replica_groups=groups,
            ins=[u_in.opt()], outs=[u_out.opt()])

        z = [dshp.tile([128, T], F32, name=f"z_{m}", tag=f"dsh{m}")
             for m in range(MSH)]
        psz = {}
        for m in range(MSH):
            for chn in range(NCH):
                psz[(m, chn)] = psmm.tile([128, TCH], F32, name="psz",
                                          tag="mm")
        for k in range(KTF):
            w2k = w1s.tile([128, DSH], BF, name="w2k", tag="w2k")
            nc.sync.dma_start(out=w2k, in_=w2[l][k * 128:(k + 1) * 128, :])
            uf = agb.tile([128, T], BF, name="uf", tag="agb")
            nc.sync.dma_start(out=uf, in_=u_out[k * 128:(k + 1) * 128, :])
            for m in range(MSH):
                for chn in range(NCH):
                    cs = slice(chn * TCH, (chn + 1) * TCH)
                    nc.tensor.matmul(psz[(m, chn)],
                                     w2k[:, m * 128:(m + 1) * 128],
                                     uf[:, cs],
                                     start=(k == 0), stop=(k == KTF - 1))
        for m in range(MSH):
            for chn in range(NCH):
                cs = slice(chn * TCH, (chn + 1) * TCH)
                nc.vector.tensor_copy(z[m][:, cs], psz[(m, chn)])
        z_in = dram.tile([DSH, T], F32, name=f"z_in{l}")
        z_out = dram.tile([TP * DSH, T], F32, name=f"z_out{l}")
        for m in range(MSH):
            nc.sync.dma_start(out=z_in[m * 128:(m + 1) * 128, :], in_=z[m])
        nc.gpsimd.collective_compute(
            "AllGather", AL.bypass, replica_groups=groups,
            ins=[z_in.opt()], outs=[z_out.opt()])
        for k in range(KT):
            zf = agf.tile([128, T], F32, name="zf", tag="agf")
            nc.sync.dma_start(out=zf, in_=z_out[k * 128:(k + 1) * 128, :])
            # x = (zf + b2) + x
            nc.vector.scalar_tensor_tensor(
                out=x[k], in0=zf, scalar=b2col[:, k:k + 1], in1=x[k],
                op0=AL.add, op1=AL.add)

    # ---------------- final LN + logits ----------------
    hf = layernorm(x, gfd[0], befd[0], "lnf")
    for n in range(NV):
        hwb = []
        for k in range(KT):
            hb = hwp.tile([128, VCH], BF, name="hwb", tag="hwb")
            nc.sync.dma_start(
                out=hb,
                in_=hwd[k * 128:(k + 1) * 128, n * VCH:(n + 1) * VCH])
            hwb.append(hb)
        for t in range(TT):
            ps = psmm.tile([128, TCH], F32, name="pslg", tag="mm")
            for k in range(KT):
                nc.tensor.matmul(ps[:, 0:VCH],
                                 hf[k][:, t * 128:(t + 1) * 128],
                                 hwb[k],
                                 start=(k == 0), stop=(k == KT - 1))
            lg = lgp.tile([128, VCH], F32, name="lg", tag="lg")
            nc.vector.tensor_copy(lg, ps[:, 0:VCH])
            nc.sync.dma_start(
                out=logits[t * 128:(t + 1) * 128, n * VCH:(n + 1) * VCH],
                in_=lg)

    ctx.close()


# ---------------- host side ----------------

_PROG_CACHE = {}


def _get_program():
    if "nc" not in _PROG_CACHE:
        _PROG_CACHE["nc"] = build_program()
    return _PROG_CACHE["nc"]


def make_in_maps(input_ids, emb, Wq, Wk, Wv, Wo, W1, b1, W2, b2,
                 ln1_g, ln1_b, ln2_g, ln2_b, lnf_g, lnf_b, head_w):
    TP = CFG["TP"]
    D, V = CFG["D"], CFG["V"]
    DSH, DFS, VSH = D // TP, 4 * D // TP, V // TP
    bf = ml_dtypes.bfloat16
    in_maps = []
    for c in range(N_CORES):
        g, r = c // TP, c % TP
        x0 = np.asarray(emb)[np.asarray(input_ids)[g]]          # [S, D] f32
        in_maps.append({
            "xT0": np.ascontiguousarray(x0.T).astype(np.float32),
            "wq": np.ascontiguousarray(Wq[:, :, r * DSH:(r + 1) * DSH]).astype(bf),
            "wk": np.ascontiguousarray(Wk[:, :, r * DSH:(r + 1) * DSH]).astype(bf),
            "wv": np.ascontiguousarray(Wv[:, :, r * DSH:(r + 1) * DSH]).astype(bf),
            "wo": np.ascontiguousarray(Wo[:, :, r * DSH:(r + 1) * DSH]).astype(bf),
            "w1": np.ascontiguousarray(W1[:, :, r * DFS:(r + 1) * DFS]).astype(bf),
            "w2": np.ascontiguousarray(W2[:, :, r * DSH:(r + 1) * DSH]).astype(bf),
            "b1": np.ascontiguousarray(b1[:, r * DFS:(r + 1) * DFS]).astype(np.float32),
            "b2": np.asarray(b2, dtype=np.float32),
            "g1": np.asarray(ln1_g, dtype=np.float32),
            "be1": np.asarray(ln1_b, dtype=np.float32),
            "g2": np.asarray(ln2_g, dtype=np.float32),
            "be2": np.asarray(ln2_b, dtype=np.float32),
            "gf": np.asarray(lnf_g, dtype=np.float32).reshape(1, -1),
            "bef": np.asarray(lnf_b, dtype=np.float32).reshape(1, -1),
            "hw": np.ascontiguousarray(
                head_w[:, r * VSH:(r + 1) * VSH]).astype(bf),
        })
    return in_maps


def kernel(**inputs):
    B, S, V = CFG["B"], CFG["S"], CFG["V"]
    TP = CFG["TP"]
    VSH = V // TP
    nc = _get_program()
    in_maps = make_in_maps(**inputs)
    res = run_bass_kernel_spmd(nc, in_maps, list(range(N_CORES)), trace=False)
    out = np.empty((B, S, V), dtype=np.float32)
    for c in range(N_CORES):
        g, r = c // TP, c % TP
        out[g, :, r * VSH:(r + 1) * VSH] = res.results[c]["logits"]
    return out


def run_traced(**inputs):
    """Like kernel() but with NTFF tracing; returns (out, exec_time_ns)."""
    nc = _get_program()
    in_maps = make_in_maps(**inputs)
    res = run_bass_kernel_spmd(nc, in_maps, list(range(N_CORES)), trace=True)
    B, S, V = CFG["B"], CFG["S"], CFG["V"]
    TP = CFG["TP"]
    VSH = V // TP
    out = np.empty((B, S, V), dtype=np.float32)
    for c in range(N_CORES):
        g, r = c // TP, c % TP
        out[g, :, r * VSH:(r + 1) * VSH] = res.results[c]["logits"]
    return out, res.exec_time_ns

